# revision 15
# baseline (speedup 1.0000x reference)
"""FFM layer (embedding lookup + field factorization) on 8 trn2 NeuronCores.

The reference's inner j-sum  e[b,f,:] = sum_j v[idx[b,f], j, :]  is a pure
function of the table row, so the host precomputes a 10-component row per
global feature id:

  comp[gid, 0:8] = (sum_j v[gid, j, :]) / sqrt(2)        (vsum')
  comp[gid, 8]   = w[gid] + w0/26 - 0.5*|sum_j v[gid,j,:]|^2   (affine)
  comp[gid, 9]   = 0                                      (pad)

so  out[b] = |sum_f vsum'[gid[b,f]]|^2 + sum_f affine[gid[b,f]] .

Everything except the final squared norm is a linear 26-way sum of table
rows, so lookups can be processed by whichever core/partition holds the
row and combined later.

Sharding: the 520000-row table is split by vocab across the 8 cores
(65000 rows/core), and inside a core across the 8 GPSIMD 16-partition
groups as 2 slabs x 4 batch-quarters.  Partition 16g+h of group
g = slab*4 + quarter holds bf16 component pair (2h, 2h+1) of its slab
(h<5; rows 32500..32503 are zero pads).  Lookups are routed host-side:
group list slot (b%1024)*K + k holds the k-th lookup of batch row b that
landed in this (core, slab); pads point at the zero row, so a plain
fixed-stride K-reduction on VectorE yields per-(core,b) partial sums.
The actual gather is gpsimd.ap_gather -- SBUF-resident per-partition
tables, ~1.4us per instruction regardless of index count (measured),
vs ~8.5ns/descriptor for any DMA-descriptor-based gather path.

Cross-core combination is a single ReduceScatter(add) over the 8 cores
of the [4096, 16] f32 partial tensor; rank c receives exactly its
512-row output slice, then computes |s|^2 + affine locally.
"""

import sys

import numpy as np

FIELD = 26
K_LAT = 8
VOCAB = 20000
TOTAL = FIELD * VOCAB      # 520000
B = 4096
NCORES = 8
BC = B // NCORES           # 512 output rows per core
P = 128

CORE_ROWS = TOTAL // NCORES    # 65000 table rows per core
SLAB = CORE_ROWS // 2          # 32500 rows per slab
NE = SLAB + 4                  # +4 zero-pad rows (gather target for pads)
ZROW = SLAB                    # local index of the zero row
KSLOT = 13                     # lookup slots per (batch row, core, slab)
BQ = 1024                      # batch rows per group (quarter of 4096)
GL = BQ * KSLOT                # 13312 = group list length
NCHUNK = 4                     # ap_gather chunks (pipelining)
CB = BQ // NCHUNK              # 256 batch rows per chunk
CL = CB * KSLOT                # 3328 idxs per chunk
ICOL = GL // 16                # 832 idx columns per partition

_TRN_REPO = "/opt/trn_rl_repo"

_cache = {}


def _build_nc():
    if _TRN_REPO not in sys.path:
        sys.path.insert(0, _TRN_REPO)
    from concourse import bacc, library_config, mybir, tile

    f32 = mybir.dt.float32
    bf16 = mybir.dt.bfloat16
    i16 = mybir.dt.int16
    Alu = mybir.AluOpType
    Ax = mybir.AxisListType

    nc = bacc.Bacc("TRN2", target_bir_lowering=False, debug=False)
    # per-slab table content: row s*5+h = component pair (2h, 2h+1) of
    # slab s, [NE, 2] bf16 (rows SLAB.. are zero)
    tab_d = nc.dram_tensor("tab", [2 * 5, NE, 2], bf16, kind="ExternalInput")
    # gather lists: idx16[16g+j, i] = local row of group-g list ordinal i*16+j
    idx_d = nc.dram_tensor("idx16", [P, ICOL], i16, kind="ExternalInput")
    # PE fold masks: sel[p, 0:4] sum-of-squares selector, sel[p, 4:8] affine
    sel_d = nc.dram_tensor("sel", [20, 8], f32, kind="ExternalInput")
    out_d = nc.dram_tensor("out", [B, 1], f32, kind="ExternalOutput")

    with tile.TileContext(nc) as tc:
        with tc.tile_pool(name="sb", bufs=1) as pool, \
             tc.tile_pool(name="vgp", bufs=2) as vpool, \
             tc.tile_pool(name="dram", bufs=1, space="DRAM") as dram:
            idx_sb = pool.tile([P, ICOL], i16, tag="idx")
            nc.sync.dma_start(out=idx_sb[:], in_=idx_d[:, :])

            tab_sb = pool.tile([P, NE, 2], bf16, tag="tab")
            # group g (partitions 16g..16g+4) gets slab s=g//4's content
            for g in range(8):
                s = g // 4
                nc.gpsimd.dma_start(
                    out=tab_sb[16 * g:16 * g + 5],
                    in_=tab_d[5 * s:5 * s + 5, :, :],
                )
            nc.gpsimd.load_library(library_config.ap_gather)

            # r[p, j, e] = sum_k over the K-block of batch row (group's
            # quarter*1024 + j), component pair element e
            r = pool.tile([P, BQ, 2], f32, tag="r")
            for c in range(NCHUNK):
                vg = vpool.tile([P, CL, 2], bf16, tag=f"vg{c % 2}")
                nc.gpsimd.ap_gather(
                    out_ap=vg[:],
                    in_ap=tab_sb[:],
                    idxs_ap=idx_sb[:, c * (ICOL // NCHUNK):
                                   (c + 1) * (ICOL // NCHUNK)],
                    channels=P,
                    num_elems=NE,
                    d=2,
                    num_idxs=CL,
                )
                nc.vector.tensor_reduce(
                    out=r[:, c * CB:(c + 1) * CB, :],
                    in_=vg[:].rearrange("p (b k) e -> p b e k", b=CB, k=KSLOT),
                    axis=Ax.X,
                    op=Alu.add,
                )

            # partials to DRAM, partition-contiguous: part[q, h, j, e]
            # = comp pair (2h, 2h+1) of batch row 1024q+j (h < 5);
            # slab 1 (partitions 64..127) folds in via DMA-accumulate
            part_t = dram.tile([4, 5, BQ, 2], f32)
            for q in range(4):
                nc.gpsimd.dma_start(
                    out=part_t[q, :, :, :],
                    in_=r[16 * q:16 * q + 5],
                )
            for q in range(4):
                nc.gpsimd.dma_start(
                    out=part_t[q, :, :, :],
                    in_=r[64 + 16 * q:64 + 16 * q + 5],
                    accum_op=Alu.add,
                )

            ar_t = dram.tile([4, 5, BQ, 2], f32)
            nc.gpsimd.collective_compute(
                "AllReduce",
                mybir.AluOpType.add,
                replica_groups=[list(range(NCORES))],
                ins=[part_t.opt()],
                outs=[ar_t.opt()],
            )

            # Every core computes all 4096 outputs (SPMD has no per-core
            # constants); the host slices rows [512c, 512(c+1)).
            # fin[5q+h, j, e] = summed comp pair (2h, 2h+1) of b=1024q+j
            fin = pool.tile([P, BQ, 2], f32, tag="fin")
            nc.sync.dma_start(
                out=fin[0:20],
                in_=ar_t[:, :, :, :].rearrange("q h j e -> (q h) j e"),
            )
            # masked PE matmuls fold h (cross-partition):
            #   sel[5q+h, m]   = (m == q and h < 4)   -> sum of squares
            #   sel[5q+h, 4+m] = (m == q and h == 4)  -> affine pair
            sel = pool.tile([P, 8], f32, tag="sel")
            nc.sync.dma_start(out=sel[0:20], in_=sel_d[:, :])

            out_all = pool.tile([P, BQ], f32, tag="oa")
            with tc.tile_pool(name="ps", bufs=1, space="PSUM") as psp:
                for c in range(4):
                    csl = slice(512 * c, 512 * (c + 1))
                    esq_c = pool.tile([P, 512], f32, tag="esqc", bufs=2)
                    nc.vector.tensor_tensor(
                        out=esq_c[0:20],
                        in0=fin[0:20].rearrange("p j e -> p (j e)")[:, csl],
                        in1=fin[0:20].rearrange("p j e -> p (j e)")[:, csl],
                        op=Alu.mult,
                    )
                    ps1 = psp.tile([4, 512], f32, tag=f"p1{c}", space="PSUM")
                    ps2 = psp.tile([4, 512], f32, tag=f"p2{c}", space="PSUM")
                    nc.tensor.matmul(
                        out=ps1[0:4],
                        lhsT=sel[0:20, 0:4],
                        rhs=esq_c[0:20],
                        start=True, stop=True,
                    )
                    nc.tensor.matmul(
                        out=ps2[0:4],
                        lhsT=sel[0:20, 4:8],
                        rhs=fin[0:20].rearrange("p j e -> p (j e)")[:, csl],
                        start=True, stop=True,
                    )
                    # ps1[q, (j e)]: sum over e; ps2[q, 2j] = affine
                    s2c = pool.tile([P, 256], f32, tag="s2c", bufs=2)
                    nc.vector.tensor_reduce(
                        out=s2c[0:4],
                        in_=ps1[0:4].rearrange("p (j e) -> p j e", e=2),
                        axis=Ax.X, op=Alu.add,
                    )
                    nc.vector.tensor_tensor(
                        out=out_all[0:4, 256 * c:256 * (c + 1)],
                        in0=s2c[0:4],
                        in1=ps2[0:4].rearrange("p (j e) -> p j e", e=2)[:, :, 0],
                        op=Alu.add,
                    )
            # out[b] for b = q*1024 + j  <- out_all[q, j]
            nc.sync.dma_start(
                out=out_d[:, :].rearrange("(q j) one -> q (j one)", q=4),
                in_=out_all[0:4],
            )
    nc.compile()
    return nc


def get_nc():
    if "nc" not in _cache:
        _cache["nc"] = _build_nc()
    return _cache["nc"]


def make_in_maps(inputs, offsets, w0, w, v):
    import ml_dtypes

    bf16 = ml_dtypes.bfloat16
    inp = np.asarray(inputs)
    gid = (inp + np.asarray(offsets)[None, :]).astype(np.int64)   # [B, F]
    v32 = np.asarray(v, dtype=np.float32).reshape(TOTAL, FIELD, K_LAT)
    vsum = v32.sum(axis=1)                                        # [TOTAL, 8]
    w0f = np.float32(np.asarray(w0, np.float32).reshape(()))
    comps = np.zeros((TOTAL, 10), dtype=np.float32)
    comps[:, :K_LAT] = vsum * np.float32(1.0 / np.sqrt(2.0))
    comps[:, K_LAT] = (
        np.asarray(w, dtype=np.float32).reshape(TOTAL)
        + w0f / np.float32(FIELD)
        - np.float32(0.5) * np.sum(vsum * vsum, axis=1)
    )
    comps_bf = comps.astype(bf16)

    # flatten lookups
    bb = np.repeat(np.arange(B, dtype=np.int64), FIELD)           # [B*F]
    gg = gid.reshape(-1)
    core = gg // CORE_ROWS
    rem = gg - core * CORE_ROWS
    slab = rem // SLAB
    local = (rem - slab * SLAB).astype(np.int64)
    q = bb // BQ
    j = bb - q * BQ                                               # b % 1024

    # occurrence rank k within each (core, slab, b) cell
    cell = (core * 2 + slab) * B + bb
    order = np.argsort(cell, kind="stable")
    sc = cell[order]
    run_start = np.r_[0, np.flatnonzero(sc[1:] != sc[:-1]) + 1]
    counts = np.diff(np.r_[run_start, sc.size])
    k = np.arange(sc.size) - np.repeat(run_start, counts)
    krank = np.empty(sc.size, dtype=np.int64)
    krank[order] = k
    keep = krank < KSLOT                                          # drop p~1e-4

    g_grp = slab * 4 + q
    pos = j * KSLOT + krank                                       # in-group pos
    prt = 16 * g_grp + (pos % 16)
    col = pos // 16

    maps = []
    for c in range(NCORES):
        m = keep & (core == c)
        idx16 = np.full((P, ICOL), ZROW, dtype=np.int16)
        idx16[prt[m], col[m]] = local[m].astype(np.int16)
        # table: row s*5+h = pair (2h, 2h+1) of this core's slab s
        tab = np.zeros((2 * 5, NE, 2), dtype=bf16)
        for s in range(2):
            blk = comps_bf[c * CORE_ROWS + s * SLAB:
                           c * CORE_ROWS + (s + 1) * SLAB]        # [SLAB, 10]
            tab[5 * s:5 * s + 5, :SLAB] = blk.reshape(SLAB, 5, 2).transpose(
                1, 0, 2)
        sel = np.zeros((20, 8), dtype=np.float32)
        for q in range(4):
            sel[5 * q:5 * q + 4, q] = 1.0
            sel[5 * q + 4, 4 + q] = 1.0
        maps.append({"tab": tab, "idx16": idx16, "sel": sel})
    return maps


def kernel(inputs, offsets, w0, w, v):
    if _TRN_REPO not in sys.path:
        sys.path.insert(0, _TRN_REPO)
    from concourse.bass_utils import run_bass_kernel_spmd

    nc = get_nc()
    in_maps = make_in_maps(inputs, offsets, w0, w, v)
    res = run_bass_kernel_spmd(nc, in_maps, list(range(NCORES)))
    # every core computes the full [B, 1]; take core c's slice
    out = np.concatenate(
        [np.asarray(res.results[i]["out"])[i * BC:(i + 1) * BC]
         for i in range(NCORES)],
        axis=0,
    )
    return out.astype(np.float32)


# revision 16
# speedup vs baseline: 3.6278x; 3.6278x over previous
"""FFM layer (embedding lookup + field-factorization) on 8 trn2 NeuronCores.

Strategy: data-parallel over batch (4096 rows -> 512/core), one packed
lookup table replicated to every core.  The reference's inner j-sum
e[b,f,:] = sum_j v[idx[b,f], j, :] is a pure function of the table row,
so the host precomputes a 9-float payload per global feature id:

  row[gid, 0:8] = (sum_j v[gid, j, :]) / sqrt(2)              (vsum')
  row[gid, 8]   = w[gid] + w0/26 - 0.5*|sum_j v[gid, j, :]|^2 (affine)

so that  out[b] = |sum_f row[gid[b,f]][0:8]|^2 + sum_f row[gid[b,f]][8].

This shrinks each gathered row from 1 KiB (26x8 v floats) to a 256 B
padded row, cutting HBM gather traffic 4x and the VectorE reduction ~26x
versus gathering raw v rows.  Lookups use the SWDGE dma_gather custom
instruction, one per field (field-local int16 indices into the field's
20000-row subtable), 512 indices per gather.  Q7 descriptor generation
(~8.5 ns/descriptor, measured) is the bottleneck; gathers for field
group g+1 overlap the (now tiny) VectorE reduction of group g, and the
index upload is chunked per group so the first gather starts early.
"""

import sys

import numpy as np

FIELD = 26
K = 8
RPAD = 64                # padded row length in f32 (256 B)
VOCAB = 20000
TOTAL = FIELD * VOCAB    # 520000
B = 4096
NCORES = 8
BC = B // NCORES         # 512 batch rows per core
P = 128
NT = BC // P             # 4 batch tiles
NSLOT = BC // 16         # 32 int16 index slots per idx partition

# field groups for gather/compute pipelining
GROUPS = [list(range(s, min(s + 7, FIELD))) for s in range(0, FIELD, 7)]

_TRN_REPO = "/opt/trn_rl_repo"

_cache = {}


def _build_nc():
    if _TRN_REPO not in sys.path:
        sys.path.insert(0, _TRN_REPO)
    from concourse import bacc, mybir, tile

    f32 = mybir.dt.float32
    i16 = mybir.dt.int16
    Alu = mybir.AluOpType
    Ax = mybir.AxisListType

    nc = bacc.Bacc("TRN2", target_bir_lowering=False, debug=False)
    # idx16[p, f, s] = int16 field-local index of batch row s*16+(p%16),
    # field f -- 16-partition wrap replicated to 128 host-side
    idx_d = nc.dram_tensor("idx16", [P, FIELD, NSLOT], i16,
                           kind="ExternalInput")
    tab_d = nc.dram_tensor("tab", [TOTAL, RPAD], f32, kind="ExternalInput")
    out_d = nc.dram_tensor("out", [BC, 1], f32, kind="ExternalOutput")

    NG = len(GROUPS)

    with tile.TileContext(nc) as tc:
        with tc.tile_pool(name="const", bufs=1) as cpool, \
             tc.tile_pool(name="vgp", bufs=2) as vpool:
            idx_sb = cpool.tile([P, FIELD, NSLOT], i16, tag="idx")

            # sgp[p, t, c, g] = sum over group g's fields of payload col c
            sgp = cpool.tile([P, NT, 9, NG], f32, tag="sgp")

            for gi, grp in enumerate(GROUPS):
                f0, gsz = grp[0], len(grp)
                # upload this group's indices just-in-time
                nc.sync.dma_start(
                    out=idx_sb[:, f0:f0 + gsz, :],
                    in_=idx_d[:, f0:f0 + gsz, :],
                )
                vg = vpool.tile([P, gsz, NT, RPAD], f32, tag=f"vg{gi % 2}")
                for j, f in enumerate(grp):
                    nc.gpsimd.dma_gather(
                        out_ap=vg[:, j],
                        in_ap=tab_d[f * VOCAB:(f + 1) * VOCAB, :],
                        idxs_ap=idx_sb[:, f, :],
                        num_idxs=BC,
                        num_idxs_reg=BC,
                        elem_size=RPAD,
                    )
                # field-reduction of the 9 payload cols for this group
                nc.vector.tensor_reduce(
                    out=sgp[:, :, :, gi],
                    in_=vg[:, :, :, 0:9].rearrange("p f t c -> p t c f"),
                    axis=Ax.X,
                    op=Alu.add,
                )

            # combine groups: s16[p, t, c] = sum_g sgp[p, t, c, g]
            s16 = cpool.tile([P, NT, 9], f32, tag="s16")
            nc.vector.tensor_reduce(
                out=s16[:], in_=sgp[:], axis=Ax.X, op=Alu.add
            )
            esq = cpool.tile([P, NT, K], f32, tag="esq")
            nc.vector.tensor_tensor(
                out=esq[:], in0=s16[:, :, 0:K], in1=s16[:, :, 0:K],
                op=Alu.mult,
            )
            s2s = cpool.tile([P, NT], f32, tag="s2s")
            nc.vector.tensor_reduce(
                out=s2s[:], in_=esq[:], axis=Ax.X, op=Alu.add
            )
            out_all = cpool.tile([P, NT], f32, tag="oa")
            nc.vector.tensor_tensor(
                out=out_all[:], in0=s2s[:], in1=s16[:, :, K], op=Alu.add
            )
            # single store: out[t*128+p] = out_all[p, t]
            nc.sync.dma_start(
                out=out_d[:, :].rearrange("(t p) one -> p (t one)", p=P),
                in_=out_all[:],
            )
    nc.compile()
    return nc


def get_nc():
    if "nc" not in _cache:
        _cache["nc"] = _build_nc()
    return _cache["nc"]


def make_in_maps(inputs, offsets, w0, w, v):
    del offsets  # folded into the per-field subtable slicing
    inp = np.asarray(inputs)
    # field-local int16 indices, wrapped: idx16[f, p, s] = inputs[s*16+p, f]
    idx16 = np.ascontiguousarray(
        inp.astype(np.int16).reshape(NCORES, BC, FIELD)
    )
    v32 = np.asarray(v, dtype=np.float32).reshape(TOTAL, FIELD, K)
    vsum = v32.sum(axis=1)                                   # [TOTAL, 8]
    w0f = np.float32(np.asarray(w0, np.float32).reshape(()))
    tab = np.zeros((TOTAL, RPAD), dtype=np.float32)
    tab[:, 0:K] = vsum * np.float32(1.0 / np.sqrt(2.0))
    tab[:, K] = (
        np.asarray(w, dtype=np.float32).reshape(TOTAL)
        + w0f / np.float32(FIELD)
        - np.float32(0.5) * np.sum(vsum * vsum, axis=1)
    )
    maps = []
    for i in range(NCORES):
        shard = idx16[i]                       # [BC, FIELD]
        wrapped = shard.reshape(NSLOT, 16, FIELD).transpose(1, 2, 0)
        # [16, FIELD, NSLOT] -> replicate to 128 partitions
        rep = np.ascontiguousarray(np.tile(wrapped, (NCORES, 1, 1)))
        maps.append({"idx16": rep, "tab": tab})
    return maps


def kernel(inputs, offsets, w0, w, v):
    if _TRN_REPO not in sys.path:
        sys.path.insert(0, _TRN_REPO)
    from concourse.bass_utils import run_bass_kernel_spmd

    nc = get_nc()
    in_maps = make_in_maps(inputs, offsets, w0, w, v)
    res = run_bass_kernel_spmd(nc, in_maps, list(range(NCORES)))
    out = np.concatenate(
        [np.asarray(res.results[i]["out"]) for i in range(NCORES)], axis=0
    )
    return out.astype(np.float32)


# revision 17
# speedup vs baseline: 3.7104x; 1.0228x over previous
"""FFM layer (embedding lookup + field-factorization) on 8 trn2 NeuronCores.

Strategy: data-parallel over batch (4096 rows -> 512/core), one packed
lookup table replicated to every core.  The reference's inner j-sum
e[b,f,:] = sum_j v[idx[b,f], j, :] is a pure function of the table row,
so the host precomputes a 9-float payload per global feature id:

  row[gid, 0:8] = (sum_j v[gid, j, :]) / sqrt(2)              (vsum')
  row[gid, 8]   = w[gid] + w0/26 - 0.5*|sum_j v[gid, j, :]|^2 (affine)

so that  out[b] = |sum_f row[gid[b,f]][0:8]|^2 + sum_f row[gid[b,f]][8].

This shrinks each gathered row from 1 KiB (26x8 v floats) to a 256 B
padded row, cutting HBM gather traffic 4x and the VectorE reduction ~26x
versus gathering raw v rows.  Lookups use the SWDGE dma_gather custom
instruction, one per field (field-local int16 indices into the field's
20000-row subtable), 512 indices per gather.  Q7 descriptor generation
(~8.5 ns/descriptor, measured) is the bottleneck; gathers for field
group g+1 overlap the (now tiny) VectorE reduction of group g, and the
index upload is chunked per group so the first gather starts early.
"""

import sys

import numpy as np

FIELD = 26
K = 8
RPAD = 64                # padded row length in f32 (256 B)
VOCAB = 20000
TOTAL = FIELD * VOCAB    # 520000
B = 4096
NCORES = 8
BC = B // NCORES         # 512 batch rows per core
P = 128
NT = BC // P             # 4 batch tiles
NSLOT = BC // 16         # 32 int16 index slots per idx partition

# field groups for gather/compute pipelining
GROUPS = [list(range(s, min(s + 7, FIELD))) for s in range(0, FIELD, 7)]

_TRN_REPO = "/opt/trn_rl_repo"

_cache = {}


def _build_nc():
    if _TRN_REPO not in sys.path:
        sys.path.insert(0, _TRN_REPO)
    from concourse import bacc, mybir, tile

    f32 = mybir.dt.float32
    i16 = mybir.dt.int16
    Alu = mybir.AluOpType
    Ax = mybir.AxisListType

    nc = bacc.Bacc("TRN2", target_bir_lowering=False, debug=False)
    # idx16[p, f, s] = int16 field-local index of batch row s*16+(p%16),
    # field f -- 16-partition wrap replicated to 128 host-side
    idx_d = nc.dram_tensor("idx16", [P, FIELD, NSLOT], i16,
                           kind="ExternalInput")
    tab_d = nc.dram_tensor("tab", [TOTAL, RPAD], f32, kind="ExternalInput")
    out_d = nc.dram_tensor("out", [BC, 1], f32, kind="ExternalOutput")

    NG = len(GROUPS)

    with tile.TileContext(nc) as tc:
        with tc.tile_pool(name="const", bufs=1) as cpool, \
             tc.tile_pool(name="vgp", bufs=2) as vpool:
            idx_sb = cpool.tile([P, FIELD, NSLOT], i16, tag="idx")
            nc.sync.dma_start(out=idx_sb[:], in_=idx_d[:, :, :])

            # sgp[p, t, c, g] = sum over group g's fields of payload col c
            sgp = cpool.tile([P, NT, 9, NG], f32, tag="sgp")

            for gi, grp in enumerate(GROUPS):
                f0, gsz = grp[0], len(grp)
                vg = vpool.tile([P, gsz, NT, RPAD], f32, tag=f"vg{gi % 2}")
                for j, f in enumerate(grp):
                    nc.gpsimd.dma_gather(
                        out_ap=vg[:, j],
                        in_ap=tab_d[f * VOCAB:(f + 1) * VOCAB, :],
                        idxs_ap=idx_sb[:, f, :],
                        num_idxs=BC,
                        num_idxs_reg=BC,
                        elem_size=RPAD,
                    )
                # field-reduction of the 9 payload cols for this group
                nc.vector.tensor_reduce(
                    out=sgp[:, :, :, gi],
                    in_=vg[:, :, :, 0:9].rearrange("p f t c -> p t c f"),
                    axis=Ax.X,
                    op=Alu.add,
                )

            # combine groups: s16[p, t, c] = sum_g sgp[p, t, c, g]
            s16 = cpool.tile([P, NT, 9], f32, tag="s16")
            nc.vector.tensor_reduce(
                out=s16[:], in_=sgp[:], axis=Ax.X, op=Alu.add
            )
            esq = cpool.tile([P, NT, K], f32, tag="esq")
            nc.vector.tensor_tensor(
                out=esq[:], in0=s16[:, :, 0:K], in1=s16[:, :, 0:K],
                op=Alu.mult,
            )
            s2s = cpool.tile([P, NT], f32, tag="s2s")
            nc.vector.tensor_reduce(
                out=s2s[:], in_=esq[:], axis=Ax.X, op=Alu.add
            )
            out_all = cpool.tile([P, NT], f32, tag="oa")
            nc.vector.tensor_tensor(
                out=out_all[:], in0=s2s[:], in1=s16[:, :, K], op=Alu.add
            )
            # single store: out[t*128+p] = out_all[p, t]
            nc.sync.dma_start(
                out=out_d[:, :].rearrange("(t p) one -> p (t one)", p=P),
                in_=out_all[:],
            )
    nc.compile()
    return nc


def get_nc():
    if "nc" not in _cache:
        _cache["nc"] = _build_nc()
    return _cache["nc"]


def make_in_maps(inputs, offsets, w0, w, v):
    del offsets  # folded into the per-field subtable slicing
    inp = np.asarray(inputs)
    # field-local int16 indices, wrapped: idx16[f, p, s] = inputs[s*16+p, f]
    idx16 = np.ascontiguousarray(
        inp.astype(np.int16).reshape(NCORES, BC, FIELD)
    )
    v32 = np.asarray(v, dtype=np.float32).reshape(TOTAL, FIELD, K)
    vsum = v32.sum(axis=1)                                   # [TOTAL, 8]
    w0f = np.float32(np.asarray(w0, np.float32).reshape(()))
    tab = np.zeros((TOTAL, RPAD), dtype=np.float32)
    tab[:, 0:K] = vsum * np.float32(1.0 / np.sqrt(2.0))
    tab[:, K] = (
        np.asarray(w, dtype=np.float32).reshape(TOTAL)
        + w0f / np.float32(FIELD)
        - np.float32(0.5) * np.sum(vsum * vsum, axis=1)
    )
    maps = []
    for i in range(NCORES):
        shard = idx16[i]                       # [BC, FIELD]
        wrapped = shard.reshape(NSLOT, 16, FIELD).transpose(1, 2, 0)
        # [16, FIELD, NSLOT] -> replicate to 128 partitions
        rep = np.ascontiguousarray(np.tile(wrapped, (NCORES, 1, 1)))
        maps.append({"idx16": rep, "tab": tab})
    return maps


def kernel(inputs, offsets, w0, w, v):
    if _TRN_REPO not in sys.path:
        sys.path.insert(0, _TRN_REPO)
    from concourse.bass_utils import run_bass_kernel_spmd

    nc = get_nc()
    in_maps = make_in_maps(inputs, offsets, w0, w, v)
    res = run_bass_kernel_spmd(nc, in_maps, list(range(NCORES)))
    out = np.concatenate(
        [np.asarray(res.results[i]["out"]) for i in range(NCORES)], axis=0
    )
    return out.astype(np.float32)
